# revision 1
# baseline (speedup 1.0000x reference)
"""DistMult bilinear scoring kernel for Trainium2 (8 NeuronCores).

scores[e] = left_emb[e] @ W[r_id[e]] @ right_emb[e]

Strategy:
  Host: stable-sort edges by relation (data-parallel shard over 8 cores),
        pad each relation bucket to 512-edge chunks, and pre-layout L/R into
        the transposed block format the PE wants (dim on partitions).
        The relation of every 512-edge chunk is baked into the (per-call
        compiled) kernel as a static weight-slice schedule.
  Device (identical program on all 8 cores), per 16384-edge unit
  ([128, 4096] f32 tiles, 2 MB DMAs):
    - DMA in Lt, Rt
    - per 512-wide quarter: one block-diagonal f32r matmul
      (W[r] per 32-row block) -> V.T in PSUM
    - DVE: Z = V.T (*) Rt elementwise
    - PE: zero-padded block-ones f32r matmul reduces each 32-block -> scores
    - ACT: copy scores PSUM->SBUF; DMA out score groups
  Host: inverse-permute scores back to the original edge order.

float32r streams through the PE at ~1/3 the cost of fp32 with tf32-like
rounding (~1.6e-4 rel err end to end). KERNEL_FP32=1 forces bit-exact fp32.
"""

import math
import os
import sys

import numpy as np

for _p in ("/opt/trn_rl_repo", "/root/.axon_site/_ro/trn_rl_repo"):
    if os.path.isdir(_p) and _p not in sys.path:
        sys.path.insert(0, _p)
        break

import concourse.bass as bass
import concourse.mybir as mybir
import concourse.tile as tile
from concourse import bacc, bass_utils

F32 = mybir.dt.float32
F32R = mybir.dt.float32r
USE_F32R = os.environ.get("KERNEL_FP32", "0") != "1"
MM_DT = F32R if USE_F32R else F32

DIM = 32
NUM_REL = 8
N_CORES = 8
CHUNK = 512                      # edges per matmul quarter (fp32 moving max)
TILE_FREE = 4096                 # free dim of a DMA unit tile
N_Q = TILE_FREE // CHUNK         # 8 quarters per unit
TILE_E = 4 * TILE_FREE           # 16384 edges per unit
CHUNKS_PER_TILE = TILE_E // CHUNK  # 32
GROUP_UNITS = 4                  # units per score-output group
GROUP_E = GROUP_UNITS * TILE_E   # 65536

_module_cache = {}
LAST_RESULTS = None  # BassKernelResults of the most recent run (for test.py)
_hooks_installed = False
_ldw_patched = False


def _patch_walrus_ldw_opt():
    """Enable walrus LDWEIGHTS dedup: consecutive matmuls with identical
    stationary operands skip the reload, which otherwise serializes ~220 ns
    per matmul on the PE queue."""
    global _ldw_patched
    if _ldw_patched:
        return
    _ldw_patched = True
    orig = bass_utils.run_command

    def patched(argv, **kw):
        argv = [
            "--enable-ldw-opt=true" if a == "--enable-ldw-opt=false" else a
            for a in argv
        ]
        return orig(argv, **kw)

    bass_utils.run_command = patched


def _ensure_profiling_hooks():
    """Make trace=True work in this container: install the NTFF profile hook
    (ctypes into libaxon_pjrt.so, same ABI trn_boot uses) and no-op the S3
    artifact upload."""
    global _hooks_installed
    if _hooks_installed:
        return
    _hooks_installed = True
    bass_utils.upload_artifacts = lambda tmpdir: str(tmpdir)
    try:
        import antenv.axon_hooks  # noqa: F401

        return
    except ImportError:
        pass
    import contextlib
    import ctypes
    import types

    hook = None
    so_path = "/opt/axon/libaxon_pjrt.so"
    if os.path.exists(so_path):
        lib = ctypes.CDLL(so_path)
        if hasattr(lib, "axon_start_nrt_profile"):
            lib.axon_start_nrt_profile.argtypes = [
                ctypes.POINTER(ctypes.c_int64),
                ctypes.c_size_t,
            ]
            lib.axon_start_nrt_profile.restype = ctypes.c_int64
            lib.axon_stop_nrt_profile.argtypes = [ctypes.c_char_p]
            lib.axon_stop_nrt_profile.restype = ctypes.c_int64

            @contextlib.contextmanager
            def _hook(output_dir, device_ids):
                import jax

                jax.devices()
                if device_ids:
                    ids = (ctypes.c_int64 * len(device_ids))(*device_ids)
                    rc = lib.axon_start_nrt_profile(ids, len(device_ids))
                else:
                    rc = lib.axon_start_nrt_profile(None, 0)
                if rc != 0:
                    raise RuntimeError(f"axon_start_nrt_profile rc={rc}")
                try:
                    yield
                finally:
                    n = lib.axon_stop_nrt_profile(str(output_dir).encode())
                    print(f"profile: {n} ntff file(s) in {output_dir}", file=sys.stderr)

            hook = _hook

    mod = types.ModuleType("antenv.axon_hooks")
    mod._hook = hook
    mod.get_axon_ntff_profile_hook = lambda: mod._hook

    def _set(h):
        mod._hook = h

    mod.set_axon_ntff_profile_hook = _set
    import antenv

    sys.modules["antenv.axon_hooks"] = mod
    antenv.axon_hooks = mod


def _quarter_variants(n_units: int, rel_sched: tuple):
    """Variant (block-diagonal W combo) of each (unit, quarter).

    Quarter h2 of unit u covers sorted chunks 32*u + 8*pb + h2 for pb=0..3,
    which form the 4 diagonal 32x32 blocks of its stationary matrix."""
    combos = []
    combo_idx = {}
    var_of = []
    for u in range(n_units):
        row = []
        for h2 in range(N_Q):
            c = tuple(rel_sched[32 * u + 8 * pb + h2] for pb in range(4))
            if c not in combo_idx:
                combo_idx[c] = len(combos)
                combos.append(c)
            row.append(combo_idx[c])
        var_of.append(row)
    return combos, var_of


def _build_module(n_units: int, rel_sched: tuple):
    """Build the single-core Bass program (same program runs on all 8 cores)."""
    _patch_walrus_ldw_opt()
    nc = bacc.Bacc(None, target_bir_lowering=False)
    n_groups = math.ceil(n_units / GROUP_UNITS)
    combos, var_of = _quarter_variants(n_units, rel_sched)
    n_var = len(combos)

    lt_d = nc.dram_tensor(
        "lt", (n_units, 128, TILE_FREE), MM_DT, kind="ExternalInput"
    )
    rt_d = nc.dram_tensor("rt", (n_units, 128, TILE_FREE), F32, kind="ExternalInput")
    w_d = nc.dram_tensor("wdiag", (128, n_var * 128), MM_DT, kind="ExternalInput")
    o_d = nc.dram_tensor("onesb", (128, 32), MM_DT, kind="ExternalInput")
    s_d = nc.dram_tensor(
        "scores", (n_groups, 128, GROUP_UNITS * 2 * CHUNK), F32, kind="ExternalOutput"
    )

    with tile.TileContext(nc) as tc:
        with (
            tc.tile_pool(name="const", bufs=1) as cpool,
            tc.tile_pool(name="io", bufs=3) as iop,
            tc.tile_pool(name="zp", bufs=12) as zp,
            tc.tile_pool(name="sp", bufs=2) as sp,
            tc.tile_pool(name="vps", bufs=3, space="PSUM") as vpool,
            tc.tile_pool(name="sps", bufs=1, space="PSUM") as spool,
        ):
            wdiag = cpool.tile([128, n_var * 128], MM_DT, name="wdiag_sb")
            nc.sync.dma_start(wdiag[:], w_d[:])
            onesb = cpool.tile([128, 32], MM_DT, name="onesb_sb")
            nc.sync.dma_start(onesb[:], o_d[:])

            state = {"s_sbuf": None}

            def flush(z_list, u_prev, hf_prev):
                # reduce + copy-out for half-unit (u_prev, hf_prev), pipelined
                # one half behind so the PE never stalls waiting on DVE
                m = u_prev % GROUP_UNITS
                if m == 0 and hf_prev == 0:
                    state["s_sbuf"] = sp.tile(
                        [128, GROUP_UNITS * 2 * CHUNK], F32, tag="s", name="s_sb"
                    )
                s_sbuf = state["s_sbuf"]
                # f32r matmuls require destination partition base 0: the 4
                # quarter-reduces land in 4 banks of one [32, 2048] PSUM tile.
                # Rows 0..4 hold scores; rows 4..32 are zeros (zero-padded
                # ones matrix) so everything is initialized.
                s_ps = spool.tile([32, 4 * CHUNK], F32, tag="sps", name="s_ps")
                for hh in range(4):
                    nc.tensor.matmul(
                        s_ps[0:32, CHUNK * hh : CHUNK * (hh + 1)],
                        onesb[:, :],
                        z_list[hh][:, :],
                        tile_position=(0, 0),
                    )
                col = CHUNK * (2 * m + hf_prev)
                for hh in range(4):
                    nc.scalar.copy(
                        s_sbuf[32 * hh : 32 * hh + 32, col : col + CHUNK],
                        s_ps[0:32, CHUNK * hh : CHUNK * (hh + 1)],
                    )
                if (m == GROUP_UNITS - 1 or u_prev == n_units - 1) and hf_prev == 1:
                    g = u_prev // GROUP_UNITS
                    cols = 2 * CHUNK * (m + 1)
                    nc.sync.dma_start(s_d[g, :, 0:cols], s_sbuf[:, 0:cols])

            pending = []
            for u in range(n_units):
                lt = iop.tile([128, TILE_FREE], MM_DT, tag="lt", name="lt_sb")
                nc.sync.dma_start(lt[:], lt_d[u])
                rt = iop.tile([128, TILE_FREE], F32, tag="rt", name="rt_sb")
                nc.sync.dma_start(rt[:], rt_d[u])

                for hf in range(2):
                    vps = []
                    for hh in range(4):
                        h2 = 4 * hf + hh
                        vp = vpool.tile([128, CHUNK], F32, tag="v", name="v_ps")
                        v = var_of[u][h2]
                        nc.tensor.matmul(
                            vp[:, :],
                            wdiag[:, 128 * v : 128 * (v + 1)],
                            lt[:, CHUNK * h2 : CHUNK * (h2 + 1)],
                            tile_position=(0, 0),
                        )
                        vps.append(vp)

                    if len(pending) > 1:
                        flush(*pending.pop(0))

                    z_list = []
                    for hh in range(4):
                        h2 = 4 * hf + hh
                        z = zp.tile([128, CHUNK], MM_DT, tag="z", name="z_sb")
                        nc.vector.tensor_tensor(
                            z[:],
                            vps[hh][:],
                            rt[:, CHUNK * h2 : CHUNK * (h2 + 1)],
                            op=mybir.AluOpType.mult,
                        )
                        z_list.append(z)
                    pending.append((z_list, u, hf))

            for p in pending:
                flush(*p)
    nc.finalize()
    return nc


def _prep_inputs(left, right, rid):
    """Sort/pad/shard/relayout on the host. Returns device arrays + recovery info."""
    E = left.shape[0]
    perm = np.argsort(rid, kind="stable")
    counts = np.bincount(rid, minlength=NUM_REL).astype(np.int64)
    starts = np.zeros(NUM_REL + 1, dtype=np.int64)
    np.cumsum(counts, out=starts[1:])

    # per-core segment length per relation, multiple of CHUNK
    seg = [
        int(math.ceil(c / (N_CORES * CHUNK))) * CHUNK if c > 0 else 0 for c in counts
    ]
    per_core_real = int(sum(seg))
    n_units = max(1, math.ceil(per_core_real / TILE_E))
    T = n_units * TILE_E

    # relation schedule of each sorted 512-chunk (identical on every core)
    rel_sched = []
    for r in range(NUM_REL):
        rel_sched += [r] * (seg[r] // CHUNK)
    rel_sched += [0] * ((T - per_core_real) // CHUNK)
    assert len(rel_sched) == T // CHUNK

    # gather index (into sorted order) for each device slot; -1 = padding
    gidx = np.full((N_CORES, T), -1, dtype=np.int64)
    off = 0
    for r in range(NUM_REL):
        s = seg[r]
        if s == 0:
            continue
        ar = np.arange(s, dtype=np.int64)
        for c in range(N_CORES):
            src = c * s + ar
            gidx[c, off : off + s] = np.where(src < counts[r], starts[r] + src, -1)
        off += s

    L_s = left[perm]
    R_s = right[perm]

    Lt = np.zeros((N_CORES, n_units, 128, TILE_FREE), np.float32)
    Rt = np.zeros((N_CORES, n_units, 128, TILE_FREE), np.float32)
    for c in range(N_CORES):
        gi = gidx[c]
        msk = gi >= 0
        Lc = np.zeros((T, DIM), np.float32)
        Rc = np.zeros((T, DIM), np.float32)
        Lc[msk] = L_s[gi[msk]]
        Rc[msk] = R_s[gi[msk]]
        # device layout: [u, 32*pb+k, 512*h2+n] = src[u*16384 + 4096*pb + 512*h2 + n, k]
        Lt[c] = (
            Lc.reshape(n_units, 4, N_Q, CHUNK, DIM)
            .transpose(0, 1, 4, 2, 3)
            .reshape(n_units, 128, TILE_FREE)
        )
        Rt[c] = (
            Rc.reshape(n_units, 4, N_Q, CHUNK, DIM)
            .transpose(0, 1, 4, 2, 3)
            .reshape(n_units, 128, TILE_FREE)
        )
    return perm, gidx, n_units, tuple(rel_sched), Lt, Rt


def _recover_scores(results, perm, gidx, n_units, E):
    T = n_units * TILE_E
    n_groups = math.ceil(n_units / GROUP_UNITS)
    scores_sorted = np.zeros(E, np.float32)
    for c in range(N_CORES):
        sc = np.asarray(results[c]["scores"], dtype=np.float32)
        # [g, 32hh+j (j<4), 512*(2m+hf)+n]
        #   -> sorted pos g*65536 + 16384m + 4096j + 2048hf + 512hh + n
        sc2 = (
            sc.reshape(n_groups, 4, 32, GROUP_UNITS, 2, CHUNK)[:, :, 0:4]
            .transpose(0, 3, 2, 4, 1, 5)
            .reshape(n_groups * GROUP_E)[:T]
        )
        gi = gidx[c]
        msk = gi >= 0
        scores_sorted[gi[msk]] = sc2[msk]
    scores = np.empty(E, np.float32)
    scores[perm] = scores_sorted
    return scores


def kernel(left_emb, right_emb, r_id, W):
    global LAST_RESULTS
    left = np.ascontiguousarray(np.asarray(left_emb, dtype=np.float32))
    right = np.ascontiguousarray(np.asarray(right_emb, dtype=np.float32))
    rid = np.asarray(r_id).astype(np.int64)
    Wn = np.asarray(W, dtype=np.float32)
    E = left.shape[0]

    perm, gidx, n_units, rel_sched, Lt, Rt = _prep_inputs(left, right, rid)

    # block-diagonal stationary-W variants, one per distinct quarter combo
    combos, _ = _quarter_variants(n_units, rel_sched)
    wdiag3 = np.zeros((len(combos), 128, 128), np.float32)
    for v, combo in enumerate(combos):
        for pb, r in enumerate(combo):
            wdiag3[v, 32 * pb : 32 * pb + 32, 32 * pb : 32 * pb + 32] = Wn[r]
    # packed [128, n_var*128] so the whole table loads in one contiguous DMA
    wdiag = np.ascontiguousarray(
        wdiag3.transpose(1, 0, 2).reshape(128, len(combos) * 128)
    )
    onesb = np.zeros((128, 32), np.float32)
    for j in range(4):
        onesb[32 * j : 32 * j + 32, j] = 1.0

    key = (n_units, rel_sched)
    if key not in _module_cache:
        _module_cache.clear()
        _module_cache[key] = _build_module(n_units, rel_sched)
    nc = _module_cache[key]

    in_maps = [
        {"lt": Lt[c], "rt": Rt[c], "wdiag": wdiag, "onesb": onesb}
        for c in range(N_CORES)
    ]
    trace = bool(int(os.environ.get("KERNEL_TRACE", "0")))
    kwargs = {}
    if trace:
        _ensure_profiling_hooks()
        tdir = os.environ.get("KERNEL_TRACE_DIR")
        if tdir:
            os.makedirs(tdir, exist_ok=True)
            kwargs["tmpdir"] = tdir
    res = bass_utils.run_bass_kernel_spmd(
        nc, in_maps, core_ids=list(range(N_CORES)), trace=trace, **kwargs
    )
    LAST_RESULTS = res
    return _recover_scores(res.results, perm, gidx, n_units, E)

